# revision 1
# baseline (speedup 1.0000x reference)
"""Trainium2 Bass kernel for the Involution module (B=4, C=64, H=W=128, K=7, G=4).

Algorithm per core (8-way data parallel: core = (batch, h-half)):
  - layout: partition p = channel c + 64*hb, hb = which 32-row half-of-half;
    free dim = zero-padded 38x134 pixel slab (halo rows included).
  - 1x1 kernel-generating conv as matmuls (K=64 contract over channels) into
    PSUM, fused BN+SiLU on ScalarE (per-partition scale/bias) -> bf16 SBUF.
  - 16x channel-replication of the per-pixel kernels with tiny-K matmuls
    (one-hot selection lhsT), ScalarE copy PSUM->SBUF bf16.
  - involution MAC: 49 shifted tensor_tensor mult/add on VectorE (bf16 2x
    mode; an element-shifted copy of x keeps all windows 4B-aligned).
"""

import os

os.environ.setdefault("JAX_PLATFORMS", "cpu")

import numpy as np
import ml_dtypes

import concourse.bacc as bacc
import concourse.tile as tile
import concourse.mybir as mybir
from concourse.bass_utils import run_bass_kernel_spmd

# Problem constants (hardcoded per harness contract).
B, C, H, W = 4, 64, 128, 128
K, G, GC = 7, 4, 16
KK = K * K
KO = KK * G  # 196
PAD = 3
BN_EPS = 1e-5

HB_ROWS = 32          # rows per half-of-half (per partition group)
SLAB_R = HB_ROWS + 6  # 38 padded rows per hb slab
SLAB_W = W + 6        # 134 padded cols
SLAB_F = SLAB_R * SLAB_W
RPC = 4               # output rows per pixel chunk (512 px)
NCHUNK = HB_ROWS // RPC  # 8 chunks
CHW = RPC * W         # 512 free elements per chunk

# M-chunking of the 196 KO channels: chunk1 = (g, k<32) -> 128 rows,
# chunk2 = (g, 32+kk) kk<17 -> 68 rows.
M1, M2 = 128, 68
K1 = 32  # k values in chunk1 per group

USE_BF16 = True


def _dt():
    return mybir.dt.bfloat16 if USE_BF16 else mybir.dt.float32


def _npdt():
    return ml_dtypes.bfloat16 if USE_BF16 else np.float32


def build_bass():
    nc = bacc.Bacc(
        "TRN2",
        target_bir_lowering=False,
        debug=False,
        enable_asserts=False,
        num_devices=8,
    )
    DT = _dt()
    f32 = mybir.dt.float32

    xq_d = nc.dram_tensor("xq", [128, SLAB_F], DT, kind="ExternalInput").ap()
    ws1_d = nc.dram_tensor("ws1", [128, M1], DT, kind="ExternalInput").ap()
    ws2_d = nc.dram_tensor("ws2", [128, M2], DT, kind="ExternalInput").ap()
    e1_d = nc.dram_tensor("e1", [M1, K1 * 64], DT, kind="ExternalInput").ap()
    e2_d = nc.dram_tensor("e2", [M2, (KK - K1) * 64], DT, kind="ExternalInput").ap()
    sc1_d = nc.dram_tensor("sc1", [M1, 1], f32, kind="ExternalInput").ap()
    sh1_d = nc.dram_tensor("sh1", [M1, 1], f32, kind="ExternalInput").ap()
    sc2_d = nc.dram_tensor("sc2", [M2, 1], f32, kind="ExternalInput").ap()
    sh2_d = nc.dram_tensor("sh2", [M2, 1], f32, kind="ExternalInput").ap()
    out_d = nc.dram_tensor("out", [128, HB_ROWS * W], f32, kind="ExternalOutput").ap()

    with tile.TileContext(nc) as tc:
        build_kernel(
            tc, xq_d, ws1_d, ws2_d, e1_d, e2_d, sc1_d, sh1_d, sc2_d, sh2_d, out_d
        )
    nc.compile()
    return nc


def build_kernel(tc, xq_d, ws1_d, ws2_d, e1_d, e2_d, sc1_d, sh1_d, sc2_d, sh2_d, out_d):
    from contextlib import ExitStack

    nc = tc.nc
    DT = _dt()
    f32 = mybir.dt.float32

    ctx = ExitStack()
    consts = ctx.enter_context(tc.tile_pool(name="consts", bufs=1))
    wwpool = ctx.enter_context(tc.tile_pool(name="ww", bufs=2))
    wxpool = ctx.enter_context(tc.tile_pool(name="wx", bufs=4))
    tmppool = ctx.enter_context(tc.tile_pool(name="tmp", bufs=3))
    accpool = ctx.enter_context(tc.tile_pool(name="acc", bufs=2))
    outpool = ctx.enter_context(tc.tile_pool(name="outf", bufs=2))
    zpool = ctx.enter_context(tc.tile_pool(name="z", bufs=1, space="PSUM"))
    wepool = ctx.enter_context(tc.tile_pool(name="wexp", bufs=4, space="PSUM"))

    xq = consts.tile([128, SLAB_F], DT)
    nc.sync.dma_start(out=xq, in_=xq_d)
    ws1 = consts.tile([128, M1], DT)
    nc.sync.dma_start(out=ws1, in_=ws1_d)
    ws2 = consts.tile([128, M2], DT)
    nc.sync.dma_start(out=ws2, in_=ws2_d)
    e1 = consts.tile([M1, K1, 64], DT)
    nc.sync.dma_start(out=e1, in_=e1_d.rearrange("p (k c) -> p k c", k=K1))
    e2 = consts.tile([M2, KK - K1, 64], DT)
    nc.sync.dma_start(out=e2, in_=e2_d.rearrange("p (k c) -> p k c", k=KK - K1))
    sc1 = consts.tile([M1, 1], f32)
    nc.sync.dma_start(out=sc1, in_=sc1_d)
    sh1 = consts.tile([M1, 1], f32)
    nc.sync.dma_start(out=sh1, in_=sh1_d)
    sc2 = consts.tile([M2, 1], f32)
    nc.sync.dma_start(out=sc2, in_=sc2_d)
    sh2 = consts.tile([M2, 1], f32)
    nc.sync.dma_start(out=sh2, in_=sh2_d)

    # Element-shifted copy of the slab so odd-dw windows stay 4B-aligned
    # (keeps the DVE in bf16 2x mode).
    if USE_BF16:
        xqo = consts.tile([128, SLAB_F], DT)
        nc.vector.tensor_copy(xqo[:, 0 : SLAB_F - 2], xq[:, 1 : SLAB_F - 1])
    else:
        xqo = None

    silu = mybir.ActivationFunctionType.Silu
    xq3 = xq.rearrange("p (r w) -> p r w", w=SLAB_W)
    xqo3 = xqo.rearrange("p (r w) -> p r w", w=SLAB_W) if xqo is not None else None

    for j in range(NCHUNK):
        # ---- 1x1 conv for this chunk's pixels (both halves) ----
        # interior window: slab rows 4j+3..4j+7, cols 3..130
        ww = {}
        for hb in range(2):
            p0 = 64 * hb
            rhs = xq3[p0 : p0 + 64, RPC * j + PAD : RPC * j + PAD + RPC, PAD : PAD + W]
            z1 = zpool.tile([128, CHW], f32, tag=f"z1{hb}")
            nc.tensor.matmul(
                z1,
                ws1[p0 : p0 + 64, :],
                rhs,
                start=True,
                stop=True,
            )
            z2 = zpool.tile([M2, CHW], f32, tag=f"z2{hb}")
            nc.tensor.matmul(
                z2,
                ws2[p0 : p0 + 64, :],
                rhs,
                start=True,
                stop=True,
            )
            # BN + SiLU on ScalarE, per-partition scale/bias -> bf16 SBUF
            w1 = wwpool.tile([128, CHW], DT, tag=f"ww1{hb}")
            nc.scalar.activation(w1, z1, silu, bias=sh1, scale=sc1)
            w2 = wwpool.tile([M2, CHW], DT, tag=f"ww2{hb}")
            nc.scalar.activation(w2, z2, silu, bias=sh2[0:M2], scale=sc2[0:M2])
            ww[hb] = (w1, w2)

        # ---- involution MAC over the 49 kernel positions ----
        # two bf16 accumulators (even/odd k) + fp32 combine: halves the
        # sequential-rounding walk of the accumulation
        acc = accpool.tile([128, CHW], DT, tag="acc")
        acc2 = accpool.tile([128, CHW], DT, tag="acc2")
        outf = outpool.tile([128, CHW], f32, tag="outf")
        for k in range(KK):
            dh, dw = k // K, k % K
            # expanded per-pixel kernel values: wexp[c + 64*hb, pix]
            wexp = wepool.tile([128, CHW], f32, tag="wexp")
            lhsT = e1[:, k, :] if k < K1 else e2[:, k - K1, :]
            for hb in range(2):
                w1g, w2g = ww[hb]
                src = w1g if k < K1 else w2g
                nc.tensor.matmul(
                    wexp[64 * hb : 64 * hb + 64, :],
                    lhsT,
                    src,
                    start=True,
                    stop=True,
                )
            wx = wxpool.tile([128, CHW], DT, tag="wx")
            nc.scalar.copy(wx, wexp)

            # shifted x window for this (dh, dw)
            r0 = RPC * j + dh
            if USE_BF16 and (dw % 2 == 1):
                xwin = xqo3[:, r0 : r0 + RPC, dw - 1 : dw - 1 + W]
            else:
                xwin = xq3[:, r0 : r0 + RPC, dw : dw + W]

            a = acc if k % 2 == 0 else acc2
            if k < 2:
                nc.vector.tensor_mul(a, xwin, wx)
            else:
                t = tmppool.tile([128, CHW], DT, tag="tmp")
                nc.vector.tensor_mul(t, xwin, wx)
                nc.vector.tensor_add(a, a, t)
        nc.vector.tensor_add(outf, acc, acc2)

        nc.sync.dma_start(out=out_d[:, j * CHW : (j + 1) * CHW], in_=outf)
    ctx.close()


def prep_inputs(x, conv_w, bn_gamma, bn_beta, bn_mean, bn_var):
    """Host-side prep: per-core padded slabs + shared weight tables."""
    npdt = _npdt()
    scale = (bn_gamma / np.sqrt(bn_var + BN_EPS)).astype(np.float32)
    shift = (bn_beta - bn_mean * scale).astype(np.float32)

    # KO index maps for the two M-chunks
    m1 = np.arange(M1)
    ko1 = (m1 // K1) * KK + (m1 % K1)
    m2 = np.arange(M2)
    ko2 = (m2 // 17) * KK + K1 + (m2 % 17)

    ws1 = np.zeros((128, M1), npdt)
    ws1[0:64] = conv_w[ko1].T.astype(npdt)
    ws1[64:128] = ws1[0:64]
    ws2 = np.zeros((128, M2), npdt)
    ws2[0:64] = conv_w[ko2].T.astype(npdt)
    ws2[64:128] = ws2[0:64]

    e1 = np.zeros((M1, K1, 64), npdt)
    for g in range(G):
        for k in range(K1):
            e1[g * K1 + k, k, g * GC : (g + 1) * GC] = 1.0
    e2 = np.zeros((M2, KK - K1, 64), npdt)
    for g in range(G):
        for kk in range(KK - K1):
            e2[g * 17 + kk, kk, g * GC : (g + 1) * GC] = 1.0

    sc1 = scale[ko1].reshape(M1, 1)
    sh1 = shift[ko1].reshape(M1, 1)
    sc2 = scale[ko2].reshape(M2, 1)
    sh2 = shift[ko2].reshape(M2, 1)

    xp = np.zeros((B, C, H + 2 * PAD, W + 2 * PAD), npdt)
    xp[:, :, PAD : PAD + H, PAD : PAD + W] = x.astype(npdt)

    in_maps = []
    for core in range(8):
        b, half = core // 2, core % 2
        h0 = 64 * half
        xq = np.zeros((128, SLAB_F), npdt)
        for hb in range(2):
            r0 = h0 + HB_ROWS * hb  # first output row of this hb (unpadded idx)
            slab = xp[b, :, r0 : r0 + SLAB_R, :]  # [64, 38, 134] (padded idx r0..)
            xq[64 * hb : 64 * hb + 64] = slab.reshape(C, SLAB_F)
        in_maps.append(
            {
                "xq": xq,
                "ws1": ws1,
                "ws2": ws2,
                "e1": e1.reshape(M1, K1 * 64),
                "e2": e2.reshape(M2, (KK - K1) * 64),
                "sc1": sc1,
                "sh1": sh1,
                "sc2": sc2,
                "sh2": sh2,
            }
        )
    return in_maps


def assemble_output(results):
    out = np.zeros((B, C, H, W), np.float32)
    for core in range(8):
        b, half = core // 2, core % 2
        h0 = 64 * half
        oc = results[core]["out"].reshape(128, HB_ROWS, W)
        for hb in range(2):
            out[b, :, h0 + HB_ROWS * hb : h0 + HB_ROWS * (hb + 1), :] = oc[
                64 * hb : 64 * hb + 64
            ]
    return out


def kernel(x, conv_w, bn_gamma, bn_beta, bn_mean, bn_var):
    x = np.asarray(x, np.float32)
    conv_w = np.asarray(conv_w, np.float32)
    in_maps = prep_inputs(
        x,
        conv_w,
        np.asarray(bn_gamma, np.float32),
        np.asarray(bn_beta, np.float32),
        np.asarray(bn_mean, np.float32),
        np.asarray(bn_var, np.float32),
    )
    nc = build_bass()
    res = run_bass_kernel_spmd(nc, in_maps, core_ids=list(range(8)))
    return assemble_output(res.results)


if __name__ == "__main__":
    rng = np.random.default_rng(0)
    ins = {
        "x": rng.standard_normal((B, C, H, W), np.float32),
        "conv_w": rng.standard_normal((KO, C), np.float32) / 8.0,
        "bn_gamma": rng.uniform(0.5, 1.5, KO).astype(np.float32),
        "bn_beta": rng.standard_normal(KO).astype(np.float32) * 0.1,
        "bn_mean": rng.standard_normal(KO).astype(np.float32) * 0.1,
        "bn_var": rng.uniform(0.5, 1.5, KO).astype(np.float32),
    }
    out = kernel(**ins)
    print("kernel output", out.shape, out.dtype, np.abs(out).sum())



# revision 2
# speedup vs baseline: 1.0488x; 1.0488x over previous
"""Trainium2 Bass kernel v2 for Involution (B=4, C=64, H=W=128, K=7, G=4).

Architecture (vs baseline): eliminates the 16x channel-replication matmuls
and the PSUM->SBUF weight copies entirely.

Per core (8-way data parallel, core = (batch, 64-row half)):
  - 1x1 kernel-generating conv as matmuls + fused BN+SiLU evac (ScalarE)
    -> w [196, 8192] bf16 (per-pixel kernels, KO in partitions).
  - SBUF->SBUF DMA relayout -> w_P2 [(g,ph) partitions, (k, 256 pix) free].
  - MAC: for each k: ONE DVE tensor_tensor mul [128, 4096] where the
    per-pixel kernel value is read via a stride-0 broadcast AP over the 16
    channels of the group (no materialized replication), then 8 identity
    matmuls accumulate the products into PSUM f32 (PE does all the adds).
  - ScalarE evac of the f32 accumulator + DMA out.

Layout: partition q = g*32 + ph, ph = 2-row pixel block; free = (cc, pix).
x slabs are stored per-block with 3-row halos (8x134) and twice (element-
shifted copy) so every shifted window keeps 4B alignment for DVE 2x mode.
"""

import os

os.environ.setdefault("JAX_PLATFORMS", "cpu")

import numpy as np
import ml_dtypes

import concourse.bacc as bacc
import concourse.tile as tile
import concourse.mybir as mybir
from concourse.bass_utils import run_bass_kernel_spmd

B, C, H, W = 4, 64, 128, 128
K, G, GC = 7, 4, 16
KK = K * K
KO = KK * G  # 196
PAD = 3
BN_EPS = 1e-5

ROWS = 64           # rows per core
PH = 32             # pixel blocks per group (2 rows each)
PL = 256            # pixels per block (2x128)
CC = GC             # 16
SLAB_R, SLAB_W = 8, 134
SLAB_F = SLAB_R * SLAB_W  # 1072
NPIX = ROWS * W     # 8192
FREE = CC * PL      # 4096
M1, M2 = 128, 68    # KO row chunks: (g, k<32), (g, 32+kk)
K1 = 32

BF = ml_dtypes.bfloat16


def build_bass():
    nc = bacc.Bacc(
        "TRN2",
        target_bir_lowering=False,
        debug=False,
        enable_asserts=False,
        num_devices=8,
    )
    dt = mybir.dt
    xc_d = nc.dram_tensor("xc", [65, NPIX], dt.bfloat16, kind="ExternalInput").ap()
    xs_d = nc.dram_tensor("xs", [128, CC * SLAB_F], dt.bfloat16, kind="ExternalInput").ap()
    xo_d = nc.dram_tensor("xo", [128, CC * SLAB_F], dt.bfloat16, kind="ExternalInput").ap()
    cw1_d = nc.dram_tensor("cw1", [65, M1], dt.bfloat16, kind="ExternalInput").ap()
    cw2_d = nc.dram_tensor("cw2", [65, M2], dt.bfloat16, kind="ExternalInput").ap()
    id_d = nc.dram_tensor("ident", [128, 128], dt.bfloat16, kind="ExternalInput").ap()
    out_d = nc.dram_tensor("out", [128, FREE], dt.bfloat16, kind="ExternalOutput").ap()

    with tile.TileContext(nc) as tc:
        build_kernel(tc, xc_d, xs_d, xo_d, cw1_d, cw2_d, id_d, out_d)
    nc.compile()
    return nc


def build_kernel(tc, xc_d, xs_d, xo_d, cw1_d, cw2_d, id_d, out_d):
    from contextlib import ExitStack

    nc = tc.nc
    dt = mybir.dt
    silu = mybir.ActivationFunctionType.Silu

    ctx = ExitStack()
    consts = ctx.enter_context(tc.tile_pool(name="consts", bufs=1))
    wpool = ctx.enter_context(tc.tile_pool(name="w", bufs=1))
    wxpool = ctx.enter_context(tc.tile_pool(name="wx", bufs=4))
    outpool = ctx.enter_context(tc.tile_pool(name="outf", bufs=2))
    xcpool = ctx.enter_context(tc.tile_pool(name="xcp", bufs=1))

    # xc first (conv input, with the BN-shift ones-row), then the conv
    # weights (BN scale/shift folded in host-side); the big x slabs stream
    # on the second HWDGE ring underneath the conv phase.
    xc = xcpool.tile([65, NPIX], dt.bfloat16)
    for t in range(4):
        nc.sync.dma_start(out=xc[:, t * 2048:(t + 1) * 2048],
                          in_=xc_d[:, t * 2048:(t + 1) * 2048])
    cw1 = consts.tile([65, M1], dt.bfloat16)
    nc.sync.dma_start(out=cw1, in_=cw1_d)
    cw2 = consts.tile([65, M2], dt.bfloat16)
    nc.sync.dma_start(out=cw2, in_=cw2_d)
    ident = consts.tile([128, 128], dt.bfloat16)
    nc.sync.dma_start(out=ident, in_=id_d)
    # big x slabs trail the SAME SP ring: FIFO dispatch guarantees the conv
    # inputs above transfer first at full bandwidth; descriptor size capped
    # so the slab packets share SDMA engines with the relayout ring
    xs = consts.tile([128, CC * SLAB_F], dt.bfloat16)
    nc.sync.dma_start(out=xs, in_=xs_d, max_dma_last_dim=2144)
    xo = consts.tile([128, CC * SLAB_F], dt.bfloat16)
    nc.sync.dma_start(out=xo, in_=xo_d, max_dma_last_dim=2144)

    # ---- 1x1 conv + BN + SiLU; M1 chunk first so the k<32 relayout (and
    # with it the MAC) can start while the M2 chunk still runs ----
    w1sb = wpool.tile([128, NPIX], dt.bfloat16)
    w2sb = wpool.tile([68, NPIX], dt.bfloat16)
    wp2 = wpool.tile([128, KK * PL], dt.bfloat16)
    src1 = w1sb.rearrange("(g k) f -> g k f", g=G)
    src2 = w2sb.rearrange("(g k) f -> g k f", g=G)

    def relayout(k):
        # One DMA per k: src = the 4 group-rows of k (partition stride
        # 32/17), dst = all 128 (g,ph) partitions at free offset k*PL.
        # Both sides keep the partition dim first (BIR verifier rule).
        src = src1[:, k, :] if k < K1 else src2[:, k - K1, :]
        nc.scalar.dma_start(out=wp2[:, k * PL:(k + 1) * PL], in_=src)

    with tc.tile_pool(name="zconv", bufs=1, space="PSUM") as zpool:
        for t in range(4):
            lo = t * 2048
            z1 = zpool.tile([128, 2048], dt.float32, tag="z1")
            for u in range(4):
                nc.tensor.matmul(
                    z1[:, u * 512:(u + 1) * 512],
                    cw1,
                    xc[:, lo + u * 512: lo + (u + 1) * 512],
                    start=True, stop=True,
                )
            nc.scalar.activation(w1sb[:, lo:lo + 2048], z1, silu)
        # first few k's relayout jumps the queue so the MAC can start the
        # moment the z1 evacs land; the z2 chunk proceeds underneath it
        for k in range(8):
            relayout(k)
        for t in range(4):
            lo = t * 2048
            z2 = zpool.tile([68, 2048], dt.float32, tag="z2")
            for u in range(4):
                nc.tensor.matmul(
                    z2[:, u * 512:(u + 1) * 512],
                    cw2,
                    xc[:, lo + u * 512: lo + (u + 1) * 512],
                    start=True, stop=True,
                )
            nc.scalar.activation(w2sb[:, lo:lo + 2048], z2, silu)
        for k in range(8, KK):
            relayout(k)

    # ---- involution MAC ----
    xs4 = xs.rearrange("p (c r w) -> p c r w", c=CC, w=SLAB_W)
    xo4 = xo.rearrange("p (c r w) -> p c r w", c=CC, w=SLAB_W)
    with tc.tile_pool(name="accp", bufs=1, space="PSUM") as apool:
        acc = apool.tile([128, FREE], dt.float32)
        for k in range(KK):
            dh, dw = k // K, k % K
            off = dh * SLAB_W + dw
            if off % 2 == 0:
                xwin = xs4[:, :, dh:dh + 2, dw:dw + 128]
            else:
                xwin = xo4[:, :, dh:dh + 2, dw - 1:dw - 1 + 128]
            wk = (
                wp2[:, k * PL:(k + 1) * PL]
                .rearrange("p (r w) -> p r w", r=2)
                .unsqueeze(1)
                .to_broadcast((128, CC, 2, 128))
            )
            wx = wxpool.tile([128, FREE], dt.bfloat16, tag="wx")
            wx4 = wx.rearrange("p (c r w) -> p c r w", c=CC, r=2)
            nc.vector.tensor_tensor(wx4, xwin, wk, mybir.AluOpType.mult)
            for j in range(8):
                nc.tensor.matmul(
                    acc[:, j * 512:(j + 1) * 512],
                    ident,
                    wx[:, j * 512:(j + 1) * 512],
                    start=(k == 0),
                    stop=(k == KK - 1),
                )
        # final evac: bf16 staging, split across ScalarE and (now idle) DVE
        for h in range(4):
            lo = h * 1024
            of = outpool.tile([128, 1024], dt.bfloat16, tag="outf")
            if h % 2 == 0:
                nc.scalar.copy(of, acc[:, lo:lo + 1024])
            else:
                nc.vector.tensor_copy(of, acc[:, lo:lo + 1024])
            nc.sync.dma_start(out=out_d[:, lo:lo + 1024], in_=of)
    ctx.close()


def prep_inputs(x, conv_w, bn_gamma, bn_beta, bn_mean, bn_var):
    """Host-side prep: per-core slabs + shared weight tables (all bf16)."""
    scale = (bn_gamma / np.sqrt(bn_var + BN_EPS)).astype(np.float32)
    shift = (bn_beta - bn_mean * scale).astype(np.float32)

    m1 = np.arange(M1)
    ko1 = (m1 // K1) * KK + (m1 % K1)
    m2 = np.arange(M2)
    ko2 = (m2 // 17) * KK + K1 + (m2 % 17)

    # BN folded into the conv weights: z' = (scale*W) x + shift, the shift
    # entering through an all-ones 65th input row.
    cw1 = np.zeros((65, M1), np.float32)
    cw1[:64] = conv_w[ko1].T * scale[ko1][None, :]
    cw1[64] = shift[ko1]
    cw2 = np.zeros((65, M2), np.float32)
    cw2[:64] = conv_w[ko2].T * scale[ko2][None, :]
    cw2[64] = shift[ko2]
    cw1 = cw1.astype(BF)
    cw2 = cw2.astype(BF)
    ident = np.eye(128, dtype=np.float32).astype(BF)

    xb = x.astype(BF)
    xp = np.zeros((B, C, H + 2 * PAD, W + 2 * PAD), BF)
    xp[:, :, PAD:PAD + H, PAD:PAD + W] = xb

    in_maps = []
    for core in range(8):
        b, half = core // 2, core % 2
        r0 = ROWS * half
        xc = np.ones((65, NPIX), BF)
        xc[:64] = xb[b, :, r0:r0 + ROWS, :].reshape(64, NPIX)
        # xs[(g,ph), cc, slab]: slab = xp rows (r0+2ph-3+PAD-3 .. +8) = r0+2ph..+8 in padded idx
        xs = np.zeros((G, PH, CC, SLAB_R, SLAB_W), BF)
        for ph in range(PH):
            rs = r0 + 2 * ph  # padded row index of slab row 0 (= image row r0+2ph-3)
            xs[:, ph] = xp[b, :, rs:rs + SLAB_R, :].reshape(G, CC, SLAB_R, SLAB_W)
        xs = xs.reshape(G * PH, CC * SLAB_F)
        xo = np.zeros_like(xs)
        xo[:, :-1] = xs[:, 1:]
        in_maps.append({
            "xc": xc, "xs": xs, "xo": xo,
            "cw1": cw1, "cw2": cw2,
            "ident": ident,
        })
    return in_maps


def assemble_output(results):
    out = np.zeros((B, C, H, W), np.float32)
    for core in range(8):
        b, half = core // 2, core % 2
        r0 = ROWS * half
        oc = results[core]["out"].astype(np.float32).reshape(G, PH, CC, 2, W)
        for g in range(G):
            for ph in range(PH):
                out[b, g * GC:(g + 1) * GC, r0 + 2 * ph:r0 + 2 * ph + 2, :] = oc[g, ph]
    return out


def kernel(x, conv_w, bn_gamma, bn_beta, bn_mean, bn_var):
    x = np.asarray(x, np.float32)
    conv_w = np.asarray(conv_w, np.float32)
    in_maps = prep_inputs(
        x, conv_w,
        np.asarray(bn_gamma, np.float32),
        np.asarray(bn_beta, np.float32),
        np.asarray(bn_mean, np.float32),
        np.asarray(bn_var, np.float32),
    )
    nc = build_bass()
    res = run_bass_kernel_spmd(nc, in_maps, core_ids=list(range(8)))
    return assemble_output(res.results)


if __name__ == "__main__":
    rng = np.random.default_rng(0)
    ins = {
        "x": rng.standard_normal((B, C, H, W), np.float32),
        "conv_w": rng.standard_normal((KO, C), np.float32) / 8.0,
        "bn_gamma": rng.uniform(0.5, 1.5, KO).astype(np.float32),
        "bn_beta": rng.standard_normal(KO).astype(np.float32) * 0.1,
        "bn_mean": rng.standard_normal(KO).astype(np.float32) * 0.1,
        "bn_var": rng.uniform(0.5, 1.5, KO).astype(np.float32),
    }
    out = kernel(**ins)
    print("kernel output", out.shape, out.dtype, np.abs(out).sum())


# revision 3
# speedup vs baseline: 1.0697x; 1.0200x over previous
"""Trainium2 Bass kernel v2 for Involution (B=4, C=64, H=W=128, K=7, G=4).

Architecture (vs baseline): eliminates the 16x channel-replication matmuls
and the PSUM->SBUF weight copies entirely.

Per core (8-way data parallel, core = (batch, 64-row half)):
  - 1x1 kernel-generating conv as matmuls + fused BN+SiLU evac (ScalarE)
    -> w [196, 8192] bf16 (per-pixel kernels, KO in partitions).
  - SBUF->SBUF DMA relayout -> w_P2 [(g,ph) partitions, (k, 256 pix) free].
  - MAC: for each k: ONE DVE tensor_tensor mul [128, 4096] where the
    per-pixel kernel value is read via a stride-0 broadcast AP over the 16
    channels of the group (no materialized replication), then 8 identity
    matmuls accumulate the products into PSUM f32 (PE does all the adds).
  - ScalarE evac of the f32 accumulator + DMA out.

Layout: partition q = g*32 + ph, ph = 2-row pixel block; free = (cc, pix).
x slabs are stored per-block with 3-row halos (8x134) and twice (element-
shifted copy) so every shifted window keeps 4B alignment for DVE 2x mode.
"""

import os

os.environ.setdefault("JAX_PLATFORMS", "cpu")

import numpy as np
import ml_dtypes

import concourse.bacc as bacc
import concourse.tile as tile
import concourse.mybir as mybir
from concourse.bass_utils import run_bass_kernel_spmd

B, C, H, W = 4, 64, 128, 128
K, G, GC = 7, 4, 16
KK = K * K
KO = KK * G  # 196
PAD = 3
BN_EPS = 1e-5

ROWS = 64           # rows per core
PH = 32             # pixel blocks per group (2 rows each)
PL = 256            # pixels per block (2x128)
CC = GC             # 16
SLAB_R, SLAB_W = 8, 134
SLAB_F = SLAB_R * SLAB_W  # 1072
NPIX = ROWS * W     # 8192
FREE = CC * PL      # 4096
M1, M2 = 128, 68    # KO row chunks: (g, k<32), (g, 32+kk)
K1 = 32

BF = ml_dtypes.bfloat16


def build_bass():
    nc = bacc.Bacc(
        "TRN2",
        target_bir_lowering=False,
        debug=False,
        enable_asserts=False,
        num_devices=8,
    )
    dt = mybir.dt
    xc_d = nc.dram_tensor("xc", [65, NPIX], dt.bfloat16, kind="ExternalInput").ap()
    xs_d = nc.dram_tensor("xs", [128, CC * SLAB_F], dt.bfloat16, kind="ExternalInput").ap()
    xo_d = nc.dram_tensor("xo", [128, CC * SLAB_F], dt.bfloat16, kind="ExternalInput").ap()
    cw1_d = nc.dram_tensor("cw1", [65, M1], dt.bfloat16, kind="ExternalInput").ap()
    cw2_d = nc.dram_tensor("cw2", [65, M2], dt.bfloat16, kind="ExternalInput").ap()
    id_d = nc.dram_tensor("ident", [128, 128], dt.bfloat16, kind="ExternalInput").ap()
    out_d = nc.dram_tensor("out", [128, FREE], dt.bfloat16, kind="ExternalOutput").ap()

    with tile.TileContext(nc) as tc:
        build_kernel(tc, xc_d, xs_d, xo_d, cw1_d, cw2_d, id_d, out_d)
    nc.compile()
    return nc


def build_kernel(tc, xc_d, xs_d, xo_d, cw1_d, cw2_d, id_d, out_d):
    from contextlib import ExitStack

    nc = tc.nc
    dt = mybir.dt
    silu = mybir.ActivationFunctionType.Silu

    ctx = ExitStack()
    consts = ctx.enter_context(tc.tile_pool(name="consts", bufs=1))
    wpool = ctx.enter_context(tc.tile_pool(name="w", bufs=1))
    wxpool = ctx.enter_context(tc.tile_pool(name="wx", bufs=4))
    outpool = ctx.enter_context(tc.tile_pool(name="outf", bufs=2))
    xcpool = ctx.enter_context(tc.tile_pool(name="xcp", bufs=1))

    # xc first (conv input, with the BN-shift ones-row), then the conv
    # weights (BN scale/shift folded in host-side); the big x slabs stream
    # on the second HWDGE ring underneath the conv phase.
    xc = xcpool.tile([65, NPIX], dt.bfloat16)
    nc.sync.dma_start(out=xc[:, 0:2048], in_=xc_d[:, 0:2048])
    cw1 = consts.tile([65, M1], dt.bfloat16)
    nc.sync.dma_start(out=cw1, in_=cw1_d)
    cw2 = consts.tile([65, M2], dt.bfloat16)
    nc.sync.dma_start(out=cw2, in_=cw2_d)
    for t in range(1, 4):
        nc.sync.dma_start(out=xc[:, t * 2048:(t + 1) * 2048],
                          in_=xc_d[:, t * 2048:(t + 1) * 2048])
    ident = consts.tile([128, 128], dt.bfloat16)
    nc.sync.dma_start(out=ident, in_=id_d)
    # xs trails the SAME SP ring: FIFO dispatch guarantees the conv inputs
    # above transfer first at full bandwidth. xo (only read by the late
    # odd-dw taps) is issued AFTER the first relayout batch further below.
    xs = consts.tile([128, CC * SLAB_F], dt.bfloat16)
    nc.sync.dma_start(out=xs, in_=xs_d, max_dma_last_dim=2144)
    xo = consts.tile([128, CC * SLAB_F], dt.bfloat16)

    # ---- 1x1 conv + BN + SiLU; M1 chunk first so the k<32 relayout (and
    # with it the MAC) can start while the M2 chunk still runs ----
    w1sb = wpool.tile([128, NPIX], dt.bfloat16)
    w2sb = wpool.tile([68, NPIX], dt.bfloat16)
    wp2 = wpool.tile([128, KK * PL], dt.bfloat16)
    src1 = w1sb.rearrange("(g k) f -> g k f", g=G)
    src2 = w2sb.rearrange("(g k) f -> g k f", g=G)

    def relayout(k, eng=None):
        # One DMA per k: src = the 4 group-rows of k (partition stride
        # 32/17), dst = all 128 (g,ph) partitions at free offset k*PL.
        # Both sides keep the partition dim first (BIR verifier rule).
        src = src1[:, k, :] if k < K1 else src2[:, k - K1, :]
        (eng or nc.scalar).dma_start(out=wp2[:, k * PL:(k + 1) * PL], in_=src)

    with tc.tile_pool(name="zconv", bufs=1, space="PSUM") as zpool:
        for t in range(4):
            lo = t * 2048
            z1 = zpool.tile([128, 2048], dt.float32, tag="z1")
            for u in range(4):
                nc.tensor.matmul(
                    z1[:, u * 512:(u + 1) * 512],
                    cw1,
                    xc[:, lo + u * 512: lo + (u + 1) * 512],
                    start=True, stop=True,
                )
            nc.scalar.activation(w1sb[:, lo:lo + 2048], z1, silu)
        # MAC tap order: even-dw taps first (they read only xs; the shifted
        # copy xo is only needed ~60us later by the odd-dw taps)
        korder = [k for k in range(KK) if (k % K) % 2 == 0] + \
                 [k for k in range(KK) if (k % K) % 2 == 1]
        # first relayout batch rides the SP ring right behind xs (the ACT
        # ring is still busy with the SiLU chain); xo transfers after them
        for k in korder[:8]:
            relayout(k, eng=nc.sync)
        with tc.tile_wait_until(0.025):
            nc.sync.dma_start(out=xo, in_=xo_d, max_dma_last_dim=2144)
        for t in range(4):
            lo = t * 2048
            z2 = zpool.tile([68, 2048], dt.float32, tag="z2")
            for u in range(4):
                nc.tensor.matmul(
                    z2[:, u * 512:(u + 1) * 512],
                    cw2,
                    xc[:, lo + u * 512: lo + (u + 1) * 512],
                    start=True, stop=True,
                )
            nc.scalar.activation(w2sb[:, lo:lo + 2048], z2, silu)
        for k in korder[8:]:
            relayout(k)

    # ---- involution MAC ----
    xs4 = xs.rearrange("p (c r w) -> p c r w", c=CC, w=SLAB_W)
    xo4 = xo.rearrange("p (c r w) -> p c r w", c=CC, w=SLAB_W)
    with tc.tile_pool(name="accp", bufs=1, space="PSUM") as apool:
        acc = apool.tile([128, FREE], dt.float32)
        korder = [k for k in range(KK) if (k % K) % 2 == 0] + \
                 [k for k in range(KK) if (k % K) % 2 == 1]
        for idx, k in enumerate(korder):
            dh, dw = k // K, k % K
            off = dh * SLAB_W + dw
            if off % 2 == 0:
                xwin = xs4[:, :, dh:dh + 2, dw:dw + 128]
            else:
                xwin = xo4[:, :, dh:dh + 2, dw - 1:dw - 1 + 128]
            wk = (
                wp2[:, k * PL:(k + 1) * PL]
                .rearrange("p (r w) -> p r w", r=2)
                .unsqueeze(1)
                .to_broadcast((128, CC, 2, 128))
            )
            wx = wxpool.tile([128, FREE], dt.bfloat16, tag="wx")
            wx4 = wx.rearrange("p (c r w) -> p c r w", c=CC, r=2)
            nc.vector.tensor_tensor(wx4, xwin, wk, mybir.AluOpType.mult)
            for j in range(8):
                nc.tensor.matmul(
                    acc[:, j * 512:(j + 1) * 512],
                    ident,
                    wx[:, j * 512:(j + 1) * 512],
                    start=(idx == 0),
                    stop=(idx == KK - 1),
                )
        # final evac: bf16 staging, split across ScalarE and (now idle) DVE
        for h in range(4):
            lo = h * 1024
            of = outpool.tile([128, 1024], dt.bfloat16, tag="outf")
            if h % 2 == 0:
                nc.scalar.copy(of, acc[:, lo:lo + 1024])
            else:
                nc.vector.tensor_copy(of, acc[:, lo:lo + 1024])
            nc.sync.dma_start(out=out_d[:, lo:lo + 1024], in_=of)
    ctx.close()


def prep_inputs(x, conv_w, bn_gamma, bn_beta, bn_mean, bn_var):
    """Host-side prep: per-core slabs + shared weight tables (all bf16)."""
    scale = (bn_gamma / np.sqrt(bn_var + BN_EPS)).astype(np.float32)
    shift = (bn_beta - bn_mean * scale).astype(np.float32)

    m1 = np.arange(M1)
    ko1 = (m1 // K1) * KK + (m1 % K1)
    m2 = np.arange(M2)
    ko2 = (m2 // 17) * KK + K1 + (m2 % 17)

    # BN folded into the conv weights: z' = (scale*W) x + shift, the shift
    # entering through an all-ones 65th input row.
    cw1 = np.zeros((65, M1), np.float32)
    cw1[:64] = conv_w[ko1].T * scale[ko1][None, :]
    cw1[64] = shift[ko1]
    cw2 = np.zeros((65, M2), np.float32)
    cw2[:64] = conv_w[ko2].T * scale[ko2][None, :]
    cw2[64] = shift[ko2]
    cw1 = cw1.astype(BF)
    cw2 = cw2.astype(BF)
    ident = np.eye(128, dtype=np.float32).astype(BF)

    xb = x.astype(BF)
    xp = np.zeros((B, C, H + 2 * PAD, W + 2 * PAD), BF)
    xp[:, :, PAD:PAD + H, PAD:PAD + W] = xb

    in_maps = []
    for core in range(8):
        b, half = core // 2, core % 2
        r0 = ROWS * half
        xc = np.ones((65, NPIX), BF)
        xc[:64] = xb[b, :, r0:r0 + ROWS, :].reshape(64, NPIX)
        # xs[(g,ph), cc, slab]: slab = xp rows (r0+2ph-3+PAD-3 .. +8) = r0+2ph..+8 in padded idx
        xs = np.zeros((G, PH, CC, SLAB_R, SLAB_W), BF)
        for ph in range(PH):
            rs = r0 + 2 * ph  # padded row index of slab row 0 (= image row r0+2ph-3)
            xs[:, ph] = xp[b, :, rs:rs + SLAB_R, :].reshape(G, CC, SLAB_R, SLAB_W)
        xs = xs.reshape(G * PH, CC * SLAB_F)
        xo = np.zeros_like(xs)
        xo[:, :-1] = xs[:, 1:]
        in_maps.append({
            "xc": xc, "xs": xs, "xo": xo,
            "cw1": cw1, "cw2": cw2,
            "ident": ident,
        })
    return in_maps


def assemble_output(results):
    out = np.zeros((B, C, H, W), np.float32)
    for core in range(8):
        b, half = core // 2, core % 2
        r0 = ROWS * half
        oc = results[core]["out"].astype(np.float32).reshape(G, PH, CC, 2, W)
        for g in range(G):
            for ph in range(PH):
                out[b, g * GC:(g + 1) * GC, r0 + 2 * ph:r0 + 2 * ph + 2, :] = oc[g, ph]
    return out


def kernel(x, conv_w, bn_gamma, bn_beta, bn_mean, bn_var):
    x = np.asarray(x, np.float32)
    conv_w = np.asarray(conv_w, np.float32)
    in_maps = prep_inputs(
        x, conv_w,
        np.asarray(bn_gamma, np.float32),
        np.asarray(bn_beta, np.float32),
        np.asarray(bn_mean, np.float32),
        np.asarray(bn_var, np.float32),
    )
    nc = build_bass()
    res = run_bass_kernel_spmd(nc, in_maps, core_ids=list(range(8)))
    return assemble_output(res.results)


if __name__ == "__main__":
    rng = np.random.default_rng(0)
    ins = {
        "x": rng.standard_normal((B, C, H, W), np.float32),
        "conv_w": rng.standard_normal((KO, C), np.float32) / 8.0,
        "bn_gamma": rng.uniform(0.5, 1.5, KO).astype(np.float32),
        "bn_beta": rng.standard_normal(KO).astype(np.float32) * 0.1,
        "bn_mean": rng.standard_normal(KO).astype(np.float32) * 0.1,
        "bn_var": rng.uniform(0.5, 1.5, KO).astype(np.float32),
    }
    out = kernel(**ins)
    print("kernel output", out.shape, out.dtype, np.abs(out).sum())
